# revision 55
# baseline (speedup 1.0000x reference)
"""Distributed sparse attention kernel for Trainium2 (8 NeuronCores).

Sharding: head-parallel. Core c owns heads [2c, 2c+1] (128 of the 1024
projection dims). Each core reads the full queries/keys (fp32) and values
(bf16), projects Q/K/V for its heads, screens all S queries per (b, h) pair
with a cheap fp32r score scan, takes the top-48 candidates, rescores those 48
exactly in fp32 and selects the exact top-38. Attention outputs are computed
for all 48 candidates; only the 38 winners are scattered (losers get an
out-of-bounds offset, which the indirect DMA silently skips). Head outputs
[T, 128] (bf16) are exchanged with an AllToAll so each core ends up with all
16 heads for its T/8 token rows, then runs the output projection locally.

Screen: importance = max_k(s) - mean_k(s). Scores are fp32r (1 PE pass).
The max-reduce is split across two engines: pairs 0..QMAJ-1 use DVE
reduce_max over the PSUM score tiles; the rest use the Activation engine's
exp-accumulate (LSE: (1/lam)*ln sum_k exp(lam*s - lam*C) >= max_k s), so DVE
and Act run concurrently. Candidate capture was validated on data: true
top-38 always ranks <= 40 under the LSE screen + fp32r noise, vs 48 kept.

Rescore: fp32 PE scores for the 48 candidates; exact max is recovered as
8*ln(max_k exp(s/8)) from the fp32 exp tiles (Act exp is also needed for the
softmax anyway); max over keys (= partitions, k-major tiles) runs on gpsimd
partition_all_reduce (SBUF-only: GPSIMD cannot touch PSUM). Q/K projections
stay fp32: their error would shift scores coherently and break selection
(min exact gap(38,39) = 2.9e-4, fp32r error ~6e-3).
"""

import math
import sys

import numpy as np

sys.path.insert(0, "/opt/trn_rl_repo")

import concourse.bass as bass
import concourse.bass_isa as bass_isa
import concourse.mybir as mybir
import concourse.tile as tile
from concourse import bacc
from concourse.masks import make_identity
from concourse.tile import add_dep_helper

F32 = mybir.dt.float32
F32R = mybir.dt.float32r
BF16 = mybir.dt.bfloat16
U32 = mybir.dt.uint32

B = 4
D = 1024
H = 16
HD = 64
H_LOC = 2          # heads per core
U = 38             # exact top-k
NCAND = 48         # screen candidates (6 rounds of max8)
N_CORES = 8
QMAJ = 4           # pairs whose screen max runs on DVE; rest use Act LSE
LAM = 2.0          # LSE sharpness
CSH = 40.0         # LSE shift: exp(LAM*s - LAM*CSH)
OOB_BIG = 0x100000


def build_nc(S=2048, n_cores=8):
    nc = bacc.Bacc("TRN2", target_bir_lowering=False, debug=False,
                   num_devices=n_cores)
    T = B * S
    NP = 512                # projection moving-dim chunk
    NQC = S // 128          # 128-query chunks per pair
    KH = min(512, S)        # scan psum strip width
    NKH = S // KH
    NCH = min(512, KH)      # scan matmul moving-dim chunk
    NKC = S // 128          # 128-key chunks
    ROWS_OUT = T // n_cores
    scale = 1.0 / math.sqrt(HD)

    # ---- I/O ----
    xqT = nc.dram_tensor("xqT", [D, T], F32, kind="ExternalInput")
    xkT = nc.dram_tensor("xkT", [D, T], F32, kind="ExternalInput")
    xvT = nc.dram_tensor("xvT", [D, T], BF16, kind="ExternalInput")
    wqT = nc.dram_tensor("wqT", [D, 128], F32, kind="ExternalInput")
    wkT = nc.dram_tensor("wkT", [D, 128], F32, kind="ExternalInput")
    wvT = nc.dram_tensor("wvT", [D, 128], BF16, kind="ExternalInput")
    bq = nc.dram_tensor("bq", [128, 1], F32, kind="ExternalInput")
    bk = nc.dram_tensor("bk", [128, 1], F32, kind="ExternalInput")
    bv = nc.dram_tensor("bv", [128, 1], F32, kind="ExternalInput")
    woT = nc.dram_tensor("woT", [D, D], BF16, kind="ExternalInput")  # full Wo.T
    boN = nc.dram_tensor("boN", [1, D], F32, kind="ExternalInput")
    boff = nc.dram_tensor("boff", [8, 1], U32, kind="ExternalInput")
    out_ext = nc.dram_tensor("out", [ROWS_OUT, D], F32, kind="ExternalOutput")

    # ---- DRAM scratch ----
    qrm = [nc.dram_tensor(f"qrm{h}", [T, HD], F32) for h in range(H_LOC)]
    ohead = nc.dram_tensor("ohead", [T, 128], BF16)
    oa2a = nc.dram_tensor("oa2a", [T, 128], BF16)

    with tile.TileContext(nc) as tc:
        with (
            tc.tile_pool(name="resident", bufs=1) as res,
            tc.tile_pool(name="consts", bufs=1) as consts,
        ):
            # constants
            ident = consts.tile([128, 128], F32)
            make_identity(nc, ident[:])
            ones_col_bf = consts.tile([128, 1], BF16)
            nc.vector.memset(ones_col_bf[:], 1.0)
            ones_row = consts.tile([1, 128], F32)
            nc.vector.memset(ones_row[:], 1.0)
            ones_row_bf = consts.tile([1, 128], BF16)
            nc.vector.memset(ones_row_bf[:], 1.0)
            lse_bias = consts.tile([128, 1], F32)
            nc.vector.memset(lse_bias[:], -LAM * CSH)
            bq_sb = consts.tile([128, 1], F32)
            bk_sb = consts.tile([128, 1], F32)
            bv_sb = consts.tile([128, 1], F32)
            nc.sync.dma_start(out=bq_sb[:], in_=bq[:])
            nc.sync.dma_start(out=bk_sb[:], in_=bk[:])
            nc.sync.dma_start(out=bv_sb[:], in_=bv[:])
            bo_sb = consts.tile([1, D], F32)
            nc.sync.dma_start(out=bo_sb[:], in_=boN[:])
            boff_sb = consts.tile([8, 1], U32)
            nc.sync.dma_start(out=boff_sb[:], in_=boff[:])

            # resident projections
            QTr = res.tile([128, T], F32R)   # screen Q (f32r)
            KTr = res.tile([128, T], F32R)   # screen K (f32r)
            KT32 = res.tile([128, T], F32)   # exact K (fp32 rescore)
            vrmT = res.tile([128, T], BF16)  # V, dims-major

            # ------------- projections + screen (merged pipeline) -------------
            # Emission order drives engine queues: Q-proj, K-proj, screen,
            # V-proj. The screen's DVE/Act reduces start as soon as K is
            # resident (~45% into the PE work) instead of after all
            # projections; V (needed only by the rescore) fills the PE gap
            # behind the screen matmuls.
            imp_all = res.tile([128, 8 * NQC], F32)
            ks_all = res.tile([128, 8], F32)   # ksum per pair (64 used rows)
            with (
                tc.tile_pool(name="wts", bufs=1) as wts,
                tc.tile_pool(name="xin", bufs=3) as xin,
                tc.tile_pool(name="pfch", bufs=3) as pfch,
                tc.tile_pool(name="vout", bufs=3) as vout,
                tc.tile_pool(name="ps_proj", bufs=2, space="PSUM") as psp,
                tc.tile_pool(name="ps_tr", bufs=1, space="PSUM") as pstr0,
                tc.tile_pool(name="ps_scan", bufs=2, space="PSUM") as pss,
                tc.tile_pool(name="ps_scanl", bufs=2, space="PSUM") as pssl,
                tc.tile_pool(name="ps_mean", bufs=1, space="PSUM") as psm,
                tc.tile_pool(name="scan_sb", bufs=2) as ssb,
            ):
                wq_sb = wts.tile([128, 8, 128], F32)
                wk_sb = wts.tile([128, 8, 128], F32)
                wv_sb = wts.tile([128, 8, 128], BF16)
                nc.sync.dma_start(out=wq_sb[:], in_=wqT[:].rearrange("(k p) m -> p k m", p=128))
                nc.sync.dma_start(out=wk_sb[:], in_=wkT[:].rearrange("(k p) m -> p k m", p=128))
                nc.sync.dma_start(out=wv_sb[:], in_=wvT[:].rearrange("(k p) m -> p k m", p=128))

                def proj_chunk(which, xsrc, w_sb, b_sb, ncol):
                    sl = slice(ncol * NP, (ncol + 1) * NP)
                    xt = xin.tile([128, 8, NP], w_sb[:].dtype,
                                  tag="xtf" if which < 2 else "xtb",
                                  bufs=3 if which < 2 else 2)
                    nc.sync.dma_start(
                        out=xt[:],
                        in_=xsrc[:, sl].rearrange("(k p) t -> p k t", p=128),
                    )
                    ps = psp.tile([128, NP], F32, tag="pp")
                    for kc in range(8):
                        nc.tensor.matmul(ps[:], lhsT=w_sb[:, kc, :], rhs=xt[:, kc, :],
                                         start=(kc == 0), stop=(kc == 7))
                    if which == 2:  # V -> vrmT (bf16, dims-major) directly
                        nc.scalar.activation(vrmT[:, sl], ps[:],
                                             mybir.ActivationFunctionType.Identity,
                                             bias=b_sb[:])
                        return
                    pf = pfch.tile([128, NP], F32, tag="pf")
                    nc.scalar.activation(pf[:], ps[:],
                                         mybir.ActivationFunctionType.Identity,
                                         bias=b_sb[:])
                    if which == 0:
                        nc.vector.tensor_copy(QTr[:, sl], pf[:])
                        for h in range(H_LOC):
                            hsl = slice(h * 64, (h + 1) * 64)
                            qt = vout.tile([128, NP // 128, 64], F32, tag="qt")
                            for j in range(NP // 128):
                                jsl = slice(j * 128, (j + 1) * 128)
                                pst = pstr0.tile([128, 64], F32, tag="pq")
                                nc.tensor.transpose(pst[:], in_=pf[hsl, jsl],
                                                    identity=ident[hsl, hsl])
                                nc.vector.tensor_copy(qt[:, j, :], pst[:])
                            nc.sync.dma_start(
                                out=qrm[h][sl, :].rearrange("(j p) f -> p j f", p=128),
                                in_=qt[:])
                    else:
                        nc.vector.tensor_copy(KT32[:, sl], pf[:])
                        nc.vector.tensor_copy(KTr[:, sl], pf[:])

                CPB = (T // NP) // B    # projection chunks per batch

                def screen_pair(pair):
                    h, b = divmod(pair, B)
                    hsl = slice(h * 64, (h + 1) * 64)
                    bsl = slice(b * S, (b + 1) * S)
                    nc.vector.reduce_sum(ks_all[hsl, pair:pair + 1], KT32[hsl, bsl],
                                         axis=mybir.AxisListType.X)
                    # f32r matmuls need a >=256 moving dim: use a 256-wide
                    # broadcast of ksum and keep only column 0 of the product
                    ksr = ssb.tile([128, 256], F32R, tag="ksr")
                    nc.vector.tensor_copy(
                        ksr[hsl, :],
                        ks_all[hsl, pair:pair + 1].to_broadcast([64, 256]))
                    psmean = ssb.tile([128, NQC], F32, tag="pmc")
                    for qc in range(NQC):
                        qsl = slice(b * S + qc * 128, b * S + (qc + 1) * 128)
                        pmt = psm.tile([128, 256], F32, tag="pm")
                        nc.tensor.matmul(pmt[:], lhsT=QTr[hsl, qsl],
                                         rhs=ksr[hsl, :], start=True, stop=True)
                        nc.vector.tensor_copy(psmean[:, qc:qc + 1], pmt[:, 0:1])
                    icol = slice(pair * NQC, (pair + 1) * NQC)
                    if pair < B:  # head 0 -> DVE, head 1 -> Act LSE
                        mcol = ssb.tile([128, NQC], F32, tag="mcol")
                        nc.vector.tensor_scalar_mul(mcol[:], psmean[:], 1.0 / S)
                        # DVE reduce_max path
                        xcol = ssb.tile([128, NKH, NQC], F32, tag="xcol")
                        for qc in range(NQC):
                            qsl = slice(b * S + qc * 128, b * S + (qc + 1) * 128)
                            for half in range(NKH):
                                ps = pss.tile([128, KH], F32, tag="sc")
                                for j in range(KH // NCH):
                                    ksl = slice(b * S + half * KH + j * NCH,
                                                b * S + half * KH + (j + 1) * NCH)
                                    nc.tensor.matmul(ps[:, j * NCH:(j + 1) * NCH],
                                                     lhsT=QTr[hsl, qsl],
                                                     rhs=KTr[hsl, ksl],
                                                     start=True, stop=True)
                                nc.vector.reduce_max(xcol[:, half, qc:qc + 1], ps[:],
                                                     axis=mybir.AxisListType.X)
                        xmax = ssb.tile([128, NQC], F32, tag="xmax")
                        if NKH > 1:
                            nc.vector.tensor_reduce(
                                xmax[:], xcol[:].rearrange("p a q -> p q a"),
                                axis=mybir.AxisListType.X, op=mybir.AluOpType.max)
                        else:
                            nc.vector.tensor_copy(xmax[:], xcol[:, 0, :])
                        nc.vector.tensor_sub(imp_all[:, icol], xmax[:], mcol[:])
                    else:
                        # Act LSE path: exp-accumulate over each KH strip.
                        # Rank by z = sum_k exp(lam(s-C)) * exp(-lam*mean):
                        # monotone in (lse - mean), so no Ln needed and the
                        # Act engine only ever runs the Exp table.
                        lsep = ssb.tile([128, NKH, NQC], F32, tag="lsep")
                        junk = ssb.tile([128, KH], BF16, tag="junk")
                        for qc in range(NQC):
                            qsl = slice(b * S + qc * 128, b * S + (qc + 1) * 128)
                            for half in range(NKH):
                                ps = pssl.tile([128, KH], F32, tag="sc")
                                for j in range(KH // NCH):
                                    ksl = slice(b * S + half * KH + j * NCH,
                                                b * S + half * KH + (j + 1) * NCH)
                                    nc.tensor.matmul(ps[:, j * NCH:(j + 1) * NCH],
                                                     lhsT=QTr[hsl, qsl],
                                                     rhs=KTr[hsl, ksl],
                                                     start=True, stop=True)
                                nc.scalar.activation(
                                    junk[:], ps[:],
                                    mybir.ActivationFunctionType.Exp,
                                    bias=lse_bias[:], scale=LAM,
                                    accum_out=lsep[:, half, qc:qc + 1])
                        lses = ssb.tile([128, NQC], F32, tag="lses")
                        if NKH > 1:
                            nc.vector.tensor_reduce(
                                lses[:], lsep[:].rearrange("p a q -> p q a"),
                                axis=mybir.AxisListType.X, op=mybir.AluOpType.add)
                        else:
                            nc.vector.tensor_copy(lses[:], lsep[:, 0, :])
                        emean = ssb.tile([128, NQC], F32, tag="emean")
                        nc.scalar.activation(emean[:], psmean[:],
                                             mybir.ActivationFunctionType.Exp,
                                             scale=-LAM / S)
                        nc.vector.tensor_tensor(imp_all[:, icol], lses[:],
                                                emean[:],
                                                op=mybir.AluOpType.mult)

                # interleave: per batch, project Q then K chunks, then emit
                # that batch's two screen pairs (one DVE, one Act/LSE) so the
                # reduce engines start ~4x earlier
                if CPB >= 1:
                    for b in range(B):
                        for ncol in range(b * CPB, (b + 1) * CPB):
                            proj_chunk(0, xqT, wq_sb, bq_sb, ncol)
                        for ncol in range(b * CPB, (b + 1) * CPB):
                            proj_chunk(1, xkT, wk_sb, bk_sb, ncol)
                        screen_pair(b)          # h=0: DVE
                        screen_pair(B + b)      # h=1: Act LSE
                else:
                    for ncol in range(T // NP):
                        proj_chunk(0, xqT, wq_sb, bq_sb, ncol)
                    for ncol in range(T // NP):
                        proj_chunk(1, xkT, wk_sb, bk_sb, ncol)
                    for b in range(B):
                        screen_pair(b)
                        screen_pair(B + b)

                # ---- V projection (needed only from the rescore onward) ----
                for ncol in range(T // NP):
                    proj_chunk(2, xvT, wv_sb, bv_sb, ncol)

            # ---------------- candidate top-48 ----------------
            idx_tok = res.tile([8, NCAND], U32)
            off_cols = res.tile([NCAND, 8], U32)
            with (
                tc.tile_pool(name="ps_tk", bufs=1, space="PSUM") as pstk,
                tc.tile_pool(name="tk_sb", bufs=1) as tksb,
            ):
                NQ8 = 8 * NQC
                impP = tksb.tile([8, S], F32)
                pst = pstk.tile([NQ8, 128], F32)
                nc.tensor.transpose(pst[:], in_=imp_all[:, 0:NQ8],
                                    identity=ident[:])
                impT = tksb.tile([NQ8, 128], F32)
                nc.scalar.copy(impT[:], pst[:])
                for pr in range(8):
                    nc.gpsimd.dma_start(
                        out=impP[pr:pr + 1, :],
                        in_=impT[pr * NQC:(pr + 1) * NQC, :],
                    )
                work = tksb.tile([8, S], F32)
                nc.vector.tensor_copy(work[:], impP[:])
                mxv = tksb.tile([8, NCAND], F32)
                idx = tksb.tile([8, NCAND], U32)
                nrounds = NCAND // 8
                for r in range(nrounds):
                    rsl = slice(r * 8, (r + 1) * 8)
                    nc.vector.max(out=mxv[:, rsl], in_=work[:])
                    nc.vector.max_index(out=idx[:, rsl], in_max=mxv[:, rsl],
                                        in_values=work[:])
                    if r < nrounds - 1:
                        nc.vector.match_replace(out=work[:], in_to_replace=mxv[:, rsl],
                                                in_values=work[:], imm_value=-1e30)
                nc.vector.tensor_tensor(idx_tok[:], idx[:],
                                        boff_sb[:].to_broadcast([8, NCAND]),
                                        op=mybir.AluOpType.add)
                for pr in range(8):
                    nc.gpsimd.dma_start(out=off_cols[:, pr:pr + 1],
                                        in_=idx_tok[pr:pr + 1, :])

            # out-projection weights: load early, overlaps the rescore phase
            wop0 = tc.tile_pool(name="wo_pool0", bufs=1)
            wop = wop0.__enter__()
            wo_sb = wop.tile([128, 8, D], BF16)
            nc.sync.dma_start(out=wo_sb[:], in_=woT[:].rearrange("(k p) m -> p k m", p=128))
            bo_bc = wop.tile([128, D], F32)
            with tc.tile_pool(name="ps_bo", bufs=1, space="PSUM") as psbo:
                for nh in range(D // 512):
                    pb = psbo.tile([128, 512], F32, tag="pb")
                    nc.tensor.matmul(pb[:], lhsT=ones_row[0:1, :],
                                     rhs=bo_sb[:, nh * 512:(nh + 1) * 512],
                                     start=True, stop=True)
                    nc.scalar.copy(bo_bc[:, nh * 512:(nh + 1) * 512], pb[:])

            # qrm (DRAM, written by DMA) is read by indirect DMA below.
            tc.strict_bb_all_engine_barrier()

            # ------------- exact rescore + attention on candidates -------------
            with (
                tc.tile_pool(name="ps_st", bufs=2, space="PSUM") as ps_st,
                tc.tile_pool(name="ps_se", bufs=1, space="PSUM") as ps_se,
                tc.tile_pool(name="ps_ot", bufs=2, space="PSUM") as ps_ot,
                tc.tile_pool(name="ps_sm", bufs=1, space="PSUM") as ps_sm,
                tc.tile_pool(name="ps_vm", bufs=1, space="PSUM") as ps_vm,
                tc.tile_pool(name="att_sb", bufs=2) as asb,
                tc.tile_pool(name="vres", bufs=4) as vres,
            ):
                for b in range(B):
                    vsb = vres.tile([128, NKC, 128], BF16, tag="vsb")
                    nc.sync.dma_start(out=vsb[:], in_=vrmT[:, b * S:(b + 1) * S],
                                      transpose=True)
                    for h in range(H_LOC):
                        pair = h * B + b
                        hsl = slice(h * 64, (h + 1) * 64)
                        off_c = off_cols[:, pair:pair + 1]
                        qsel = asb.tile([NCAND, HD], F32, tag="qsel")
                        nc.gpsimd.indirect_dma_start(
                            out=qsel[:], out_offset=None,
                            in_=qrm[h][:],
                            in_offset=bass.IndirectOffsetOnAxis(ap=off_c, axis=0),
                        )
                        pq = ps_sm.tile([128, NCAND], F32, tag="sm")
                        nc.tensor.transpose(pq[0:64, :], in_=qsel[:],
                                            identity=ident[0:NCAND, 0:NCAND])
                        qselT = asb.tile([128, NCAND], F32, tag="qselT")
                        if h == 0:
                            nc.vector.tensor_copy(qselT[0:64, :], pq[0:64, :])
                        else:
                            # transpose outputs must start at partition 0;
                            # DMA shifts the block up to the head's base
                            qtmp = asb.tile([64, NCAND], F32, tag="qtmp")
                            nc.vector.tensor_copy(qtmp[:], pq[0:64, :])
                            nc.sync.dma_start(out=qselT[hsl, :], in_=qtmp[:])
                        # exact fp32 scores -> exp tiles; Pool finds max(exp)
                        expF = asb.tile([128, NKC, NCAND], F32, tag="expF")
                        expB = asb.tile([128, NKC, NCAND], BF16, tag="expB")
                        xall = asb.tile([128, NKC, NCAND], F32, tag="xall")
                        for kc in range(NKC):
                            ksl = slice(b * S + kc * 128, b * S + (kc + 1) * 128)
                            pstc = ps_st.tile([128, NCAND], F32, tag="st")
                            nc.tensor.matmul(pstc[:], lhsT=KT32[hsl, ksl],
                                             rhs=qselT[hsl, :], start=True, stop=True)
                            nc.scalar.activation(expF[:, kc, :], pstc[:],
                                                 mybir.ActivationFunctionType.Exp,
                                                 scale=scale)
                            nc.vector.tensor_copy(expB[:, kc, :], expF[:, kc, :])
                        nc.gpsimd.partition_all_reduce(
                            xall[:], expF[:], channels=128,
                            reduce_op=bass_isa.ReduceOp.max)
                        maxe = asb.tile([128, NCAND], F32, tag="maxe")
                        nc.vector.tensor_reduce(
                            maxe[:], xall[:].rearrange("p k c -> p c k"),
                            axis=mybir.AxisListType.X, op=mybir.AluOpType.max)
                        # rank by z = max(exp(s/8)) * exp(-mean/8): monotone in
                        # (max - mean), exact in fp32, and avoids the Ln table
                        psmr = ps_vm.tile([1, NCAND], F32, tag="vm2")
                        nc.tensor.matmul(psmr[:], lhsT=ks_all[hsl, pair:pair + 1],
                                         rhs=qselT[hsl, :], start=True, stop=True)
                        emeanx = asb.tile([1, NCAND], F32, tag="emeanx")
                        nc.scalar.activation(emeanx[:], psmr[:],
                                             mybir.ActivationFunctionType.Exp,
                                             scale=-scale / S)
                        imp_ex = asb.tile([1, NCAND], F32, tag="impex")
                        nc.vector.tensor_tensor(imp_ex[:], maxe[0:1, :],
                                                emeanx[:],
                                                op=mybir.AluOpType.mult)
                        # exact top-38 threshold via 5 rounds of max8
                        work2 = asb.tile([1, NCAND], F32, tag="work2")
                        nc.vector.tensor_copy(work2[:], imp_ex[:])
                        mxw = asb.tile([1, 40], F32, tag="mxw")
                        for r in range(5):
                            rsl = slice(r * 8, (r + 1) * 8)
                            nc.vector.max(out=mxw[:, rsl], in_=work2[:])
                            if r < 4:
                                nc.vector.match_replace(
                                    out=work2[:], in_to_replace=mxw[:, rsl],
                                    in_values=work2[:], imm_value=-1e30)
                        # winner mask -> offsets (losers get OOB, skipped)
                        mask = asb.tile([1, NCAND], U32, tag="mask")
                        nc.vector.tensor_tensor(
                            mask[:], imp_ex[:],
                            mxw[0:1, U - 1:U].to_broadcast([1, NCAND]),
                            op=mybir.AluOpType.is_ge)
                        mask_c = asb.tile([NCAND, 1], U32, tag="maskc")
                        nc.gpsimd.dma_start(out=mask_c[:], in_=mask[:])
                        m0 = asb.tile([NCAND, 1], U32, tag="m0")
                        nc.vector.tensor_scalar(out=m0[:], in0=mask_c[:],
                                                scalar1=1, scalar2=None,
                                                op0=mybir.AluOpType.bitwise_xor)
                        nc.vector.tensor_scalar(out=m0[:], in0=m0[:],
                                                scalar1=OOB_BIG, scalar2=None,
                                                op0=mybir.AluOpType.mult)
                        offs_c = asb.tile([NCAND, 1], U32, tag="offsc")
                        nc.vector.tensor_tensor(offs_c[:], off_c, m0[:],
                                                op=mybir.AluOpType.add)
                        # softmax-weighted attention output for all candidates
                        pse = ps_se.tile([NCAND, 1], F32, tag="se")
                        pot = ps_ot.tile([64, NCAND], F32, tag="ot")
                        for kc in range(NKC):
                            nc.tensor.matmul(pse[:], lhsT=expB[:, kc, :],
                                             rhs=ones_col_bf[:],
                                             start=(kc == 0), stop=(kc == NKC - 1))
                            nc.tensor.matmul(pot[:], lhsT=vsb[:, kc, hsl],
                                             rhs=expB[:, kc, :],
                                             start=(kc == 0), stop=(kc == NKC - 1))
                        se = asb.tile([NCAND, 1], F32, tag="se_sb")
                        nc.vector.tensor_scalar_add(se[:], pse[:], 1e-8)
                        rec = asb.tile([NCAND, 1], F32, tag="rec")
                        nc.vector.reciprocal(rec[:], se[:])
                        oT = asb.tile([64, NCAND], F32, tag="oT")
                        nc.vector.tensor_copy(oT[:], pot[:])
                        po = ps_sm.tile([NCAND, 64], F32, tag="sm")
                        nc.tensor.transpose(po[:], in_=oT[:], identity=ident[0:64, 0:64])
                        osel_bf = asb.tile([NCAND, HD], BF16, tag="oselbf")
                        nc.vector.tensor_scalar(out=osel_bf[:], in0=po[:],
                                                scalar1=rec[:, 0:1], scalar2=None,
                                                op0=mybir.AluOpType.mult)

                        # default rows: mean of V over keys, replicated x NKC
                        pvm = ps_vm.tile([1, 64], F32, tag="vm")
                        for kc in range(NKC):
                            nc.tensor.matmul(pvm[:], lhsT=ones_col_bf[:],
                                             rhs=vsb[:, kc, hsl],
                                             start=(kc == 0), stop=(kc == NKC - 1))
                        vmr_bf = asb.tile([1, 64], BF16, tag="vmr")
                        nc.vector.tensor_scalar_mul(vmr_bf[:], pvm[:], 1.0 / S)
                        pbc = ps_sm.tile([128, 64], F32, tag="sm")
                        nc.tensor.matmul(pbc[:], lhsT=ones_row_bf[0:1, :],
                                         rhs=vmr_bf[:], start=True, stop=True)
                        bc_big = asb.tile([128, NKC, 64], BF16, tag="bcbig")
                        for sc in range(NKC):
                            nc.vector.tensor_copy(bc_big[:, sc, :], pbc[:])
                        dfl = nc.sync.dma_start(
                            out=ohead[b * S:(b + 1) * S,
                                      h * 64:(h + 1) * 64].rearrange(
                                "(j p) f -> p j f", p=128),
                            in_=bc_big[:])
                        scat = nc.gpsimd.indirect_dma_start(
                            out=ohead[:],
                            out_offset=bass.IndirectOffsetOnAxis(
                                ap=offs_c[:, 0:1], axis=0),
                            in_=osel_bf[:], in_offset=None,
                            element_offset=h * 64,
                            bounds_check=T - 1, oob_is_err=False,
                        )
                        add_dep_helper(scat.ins, dfl.ins, sync=True,
                                       reason="scatter after default fill")

            tc.strict_bb_all_engine_barrier()
            # ---------------- exchange + output projection ----------------
            nc.gpsimd.collective_compute(
                "AllToAll",
                mybir.AluOpType.bypass,
                replica_groups=[list(range(n_cores))],
                ins=[ohead[:]],
                outs=[oa2a[:]],
            )
            NRC = ROWS_OUT // 128
            with (
                tc.tile_pool(name="fin", bufs=3) as fin,
                tc.tile_pool(name="ps_op", bufs=3, space="PSUM") as psop,
            ):
                # all hh transposes up front: queue-parallel, matmuls never wait
                hh_all = wop.tile([128, n_cores, NRC, 128], BF16)
                for src in range(n_cores):
                    nc.sync.dma_start(
                        out=hh_all[:, src, :, :],
                        in_=oa2a[src * ROWS_OUT:(src + 1) * ROWS_OUT, :],
                        transpose=True)
                for rc in range(NRC):
                    po2 = psop.tile([128, D], F32, tag="po2")
                    for nh in range(D // 512):
                        nsl = slice(nh * 512, (nh + 1) * 512)
                        for src in range(n_cores):
                            nc.tensor.matmul(po2[:, nsl],
                                             lhsT=hh_all[:, src, rc, :],
                                             rhs=wo_sb[:, src, nsl],
                                             start=(src == 0),
                                             stop=(src == n_cores - 1))
                    ft = fin.tile([128, D], F32, tag="ft")
                    nc.vector.tensor_add(ft[:], po2[:], bo_bc[:])
                    nc.sync.dma_start(out=out_ext[rc * 128:(rc + 1) * 128, :], in_=ft[:])

            wop0.__exit__(None, None, None)

    nc.finalize()
    return nc


def _prep_host_inputs(queries, keys, values, Wq, bq, Wk, bk, Wv, bv, Wo, bo,
                      S, n_cores):
    import ml_dtypes
    T = B * S
    xqT = np.ascontiguousarray(queries.reshape(T, D).T.astype(np.float32))
    xkT = np.ascontiguousarray(keys.reshape(T, D).T.astype(np.float32))
    xvT = np.ascontiguousarray(values.reshape(T, D).T.astype(ml_dtypes.bfloat16))
    woT_full = np.ascontiguousarray(Wo.T.astype(ml_dtypes.bfloat16))
    boff = (np.array([(p % B) * S for p in range(8)], np.uint32)
            .reshape(8, 1))
    in_maps = []
    for c in range(n_cores):
        rsl = slice(c * 128, (c + 1) * 128)
        in_maps.append({
            "xqT": xqT, "xkT": xkT, "xvT": xvT,
            "wqT": np.ascontiguousarray(Wq[rsl, :].T.astype(np.float32)),
            "wkT": np.ascontiguousarray(Wk[rsl, :].T.astype(np.float32)),
            "wvT": np.ascontiguousarray(Wv[rsl, :].T.astype(ml_dtypes.bfloat16)),
            "bq": bq[rsl].reshape(128, 1).astype(np.float32),
            "bk": bk[rsl].reshape(128, 1).astype(np.float32),
            "bv": bv[rsl].reshape(128, 1).astype(np.float32),
            "woT": woT_full,
            "boN": bo.reshape(1, D).astype(np.float32),
            "boff": boff,
        })
    return in_maps


_LAST_RESULT = None


def assemble_out(core_outs, S, n_cores):
    """Core c owns the contiguous token rows [c*T/n, (c+1)*T/n)."""
    return np.concatenate([np.asarray(o, np.float32) for o in core_outs],
                          axis=0).reshape(B, S, D)


def kernel(queries, keys, values, Wq, bq, Wk, bk, Wv, bv, Wo, bo):
    global _LAST_RESULT
    from concourse.bass_utils import run_bass_kernel_spmd

    queries, keys, values = (np.asarray(t, np.float32) for t in
                             (queries, keys, values))
    Wq, bq, Wk, bk, Wv, bv, Wo, bo = (np.asarray(t, np.float32) for t in
                                      (Wq, bq, Wk, bk, Wv, bv, Wo, bo))
    S = queries.shape[1]
    n_cores = N_CORES
    nc = build_nc(S=S, n_cores=n_cores)
    in_maps = _prep_host_inputs(queries, keys, values, Wq, bq, Wk, bk, Wv, bv,
                                Wo, bo, S, n_cores)
    res = run_bass_kernel_spmd(nc, in_maps, core_ids=list(range(n_cores)))
    _LAST_RESULT = res
    out = np.concatenate([res.results[c]["out"] for c in range(n_cores)], axis=0)
    return out.reshape(B, S, D).astype(np.float32)
